# revision 19
# baseline (speedup 1.0000x reference)
"""BERT self-attention (B=8, S=1024, D=768, H=12) on 8 TRN2 NeuronCores.

Strategy
--------
Data-parallel over batch: core b handles batch element b (no collectives).

Per core, everything is computed in a "transposed" layout that keeps the
tensor engine's contraction dimension in the partition axis:

  1. mixedT[e, s] = sum_d W^T[d, e] * x^T[d, s] + bias[e] in fp32r matmuls
     (full fp32 inputs); the psum evacuation adds the per-partition bias
     and rounds to bf16 for the attention stage.
  2. Q=K=V => the score matrix is symmetric: scores[t, s] = scores[s, t].
     The exp'd score tile in [t, s] layout therefore equals the transposed
     (unnormalized) probability matrix needed as the moving operand of the
     context matmul -- no probability transposes at all.
     scores tile = (MIX chunk)^T @ Z_h where Z_h zero-masks the other head
     of the pair (kills cross-head terms, keeps K=128 partition-aligned
     bf16 matmuls at 1 cycle/column).
     U = exp(0.125 * scores + mask[t]) fused into the ACT psum evacuation;
     the instruction's accum_out gives the row sums, which (scores being
     symmetric and the attention mask identically zero per the problem
     spec) equal the softmax denominators for the matching s-chunk.
  3. ctx'^T[dh, s] accumulates in fp32 psum over the eight t-chunks with
     stationary xl [128, 64], moving U.
  4. PE-transpose of ctx'^T 128-column chunks gives ctx[s, dh]; the
     denominator reciprocal (computed early, off the critical path) is
     applied as a per-partition scalar during the psum evacuation; DMA to
     the output in natural [s, d] layout.

The pair loop is software-pipelined: the projection/prep work of pair j+1
is emitted right after the attention of pair j so the tensor engine fills
the bubbles of the ACT-paced softmax pipeline.
"""

import numpy as np

import concourse.bacc as bacc
import concourse.tile as tile
from concourse import mybir
from concourse.bass_utils import run_bass_kernel_spmd
from concourse.masks import make_identity

B, S, D = 8, 1024, 768
H, DH = 12, 64
NP = 6            # e-tile pairs (2 heads each)
NT = 8            # t-chunks / s-chunks of 128
F32 = mybir.dt.float32
F32R = mybir.dt.float32r
BF16 = mybir.dt.bfloat16
EXP = mybir.ActivationFunctionType.Exp

_CACHED_NC = None


def build_nc():
    nc = bacc.Bacc("TRN2", target_bir_lowering=False)

    xT = nc.dram_tensor("xT", [D, S], BF16, kind="ExternalInput")
    wT = nc.dram_tensor("wT", [D, D], BF16, kind="ExternalInput")
    bias_d = nc.dram_tensor("bias_d", [128, NP], F32, kind="ExternalInput")
    mask_d = nc.dram_tensor("mask_d", [128, NT], F32, kind="ExternalInput")
    out_d = nc.dram_tensor("out", [S, D], F32, kind="ExternalOutput")

    with tile.TileContext(nc) as tc:
        with (
            tc.tile_pool(name="consts", bufs=1) as consts,
            tc.tile_pool(name="big", bufs=1) as big,
            tc.tile_pool(name="upool", bufs=20) as upool,
            tc.tile_pool(name="ctpool", bufs=3) as ctpool,
            tc.tile_pool(name="rpool", bufs=18) as rpool,
            tc.tile_pool(name="ps_s", bufs=2, space="PSUM") as ps_s,
            tc.tile_pool(name="ps_c", bufs=1, space="PSUM") as ps_c,
            tc.tile_pool(name="ps_t", bufs=2, space="PSUM") as ps_t,
        ):
            ident32 = consts.tile([128, 128], F32)
            make_identity(nc, ident32)
            identbf = consts.tile([128, 128], BF16)
            make_identity(nc, identbf)
            wts = big.tile([128, NP, D], BF16)
            xts = big.tile([128, NP, S], BF16)
            for k in range(NP):
                eng = nc.sync if k % 2 == 0 else nc.scalar
                eng.dma_start(out=xts[:, k, :],
                              in_=xT[k * 128:(k + 1) * 128, :])
                nc.gpsimd.dma_start(out=wts[:, k, :],
                                    in_=wT[k * 128:(k + 1) * 128, :])
            bias_t = consts.tile([128, NP], F32)
            nc.gpsimd.dma_start(out=bias_t, in_=bias_d[:, :])
            mask_t = consts.tile([128, NT], F32)
            nc.gpsimd.dma_start(out=mask_t, in_=mask_d[:, :])

            # Preload the ACT exp table while the inputs stream in.
            warm = consts.tile([128, 16], F32)
            nc.scalar.activation(out=warm, in_=ident32[:, 0:16],
                                 func=EXP, scale=0.125)

            mixbf = big.tile([128, NP, S], BF16)
            stages = [big.tile([128, H, DH], F32, name=f"stage{sj}")
                      for sj in range(NT)]

            # Persistent ping-pong Z tiles; zero halves are set once.
            zt = [[big.tile([128, S], BF16, name=f"z{q}{p}") for p in range(2)]
                  for q in range(2)]
            xlt = [[big.tile([128, NT, DH + 1], BF16, name=f"xl{q}{p}")
                    for p in range(2)] for q in range(2)]
            for q in range(2):
                olo = (1 - q) * 64
                for p in range(2):
                    nc.vector.memset(zt[q][p][olo:olo + 64, :], 0.0)
                    nc.vector.memset(xlt[q][p], 1.0)

            def prep(j):
                """Projection + Z/xl staging for head pair j."""
                pp = j % 2
                if j == 0:
                    # Pair 0 is on the critical path: run both halves
                    # concurrently (second half borrows the idle ctx slot)
                    # so the projection tracks the input DMA arrivals.
                    pms = [ps_s.tile([128, 512], F32, name="pm", bufs=1),
                           ps_c.tile([128, 512], F32, name="pc")]
                    for k in range(NP):
                        for n in range(2):
                            nc.tensor.matmul(
                                pms[n],
                                lhsT=wts[:, k, j * 128:(j + 1) * 128],
                                rhs=xts[:, k, n * 512:(n + 1) * 512],
                                start=(k == 0),
                                stop=(k == NP - 1),
                            )
                    for n in range(2):
                        nc.vector.tensor_scalar_add(
                            mixbf[:, j, n * 512:(n + 1) * 512], pms[n],
                            bias_t[:, j:j + 1]
                        )
                else:
                    for n in range(2):
                        pm = ps_s.tile([128, 512], F32, name="pm", bufs=1)
                        for k in range(NP):
                            nc.tensor.matmul(
                                pm,
                                lhsT=wts[:, k, j * 128:(j + 1) * 128],
                                rhs=xts[:, k, n * 512:(n + 1) * 512],
                                start=(k == 0),
                                stop=(k == NP - 1),
                            )
                        nc.vector.tensor_scalar_add(
                            mixbf[:, j, n * 512:(n + 1) * 512], pm,
                            bias_t[:, j:j + 1]
                        )
                zs = []
                for q in range(2):
                    z = zt[q][pp]
                    lo = q * 64
                    for n in range(2):
                        nc.vector.tensor_copy(
                            out=z[lo:lo + 64, n * 512:(n + 1) * 512],
                            in_=mixbf[lo:lo + 64, j, n * 512:(n + 1) * 512],
                        )
                    zs.append(z)
                xlns = [xlt[0][pp], xlt[1][pp]]
                for i in range(NT):
                    pt = ps_t.tile([128, 128], BF16, name="pt")
                    nc.tensor.transpose(
                        pt, mixbf[:, j, i * 128:(i + 1) * 128], identbf
                    )
                    for q in range(2):
                        nc.vector.tensor_copy(
                            out=xlns[q][:, i, 0:DH], in_=pt[:, q * 64:q * 64 + 64]
                        )
                return zs, xlns

            def scores_phase(j, q, zs, xlns=None, pcs=None):
                """Scores + exp for head (j, q); returns the U tiles. When
                pcs is given (final head), the ctx matmuls are interleaved
                so only the epilogue remains after the last exp."""
                us = []
                for i in range(NT):
                    psc = ps_s.tile([128, S], F32, name="psc")
                    for n in range(2):
                        nc.tensor.matmul(
                            psc[:, n * 512:(n + 1) * 512],
                            lhsT=zs[q][:, i * 128:(i + 1) * 128],
                            rhs=mixbf[:, j, n * 512:(n + 1) * 512],
                            start=True,
                            stop=True,
                        )
                    u = upool.tile([128, S], BF16, name="u")
                    nc.scalar.activation(
                        out=u, in_=psc, func=EXP,
                        bias=mask_t[:, i:i + 1], scale=0.125,
                    )
                    us.append(u)
                    if pcs is not None:
                        for n in range(2):
                            nc.tensor.matmul(
                                pcs[n],
                                lhsT=xlns[q][:, i, :],
                                rhs=u[:, n * 512:(n + 1) * 512],
                                start=(i == 0),
                                stop=(i == NT - 1),
                            )
                return us

            def ctx_epilogue(h, n, pc):
                """Evacuate one ctx half: transpose + normalization + stage."""
                ct = ctpool.tile([DH + 1, 512], BF16, name="ct")
                nc.vector.tensor_copy(out=ct, in_=pc)
                for sjh in range(NT // 2):
                    sj = n * 4 + sjh
                    po = ps_t.tile([128, DH + 1], BF16, name="pt")
                    nc.tensor.transpose(
                        po,
                        ct[:, sjh * 128:(sjh + 1) * 128],
                        identbf[0:DH + 1, 0:DH + 1],
                    )
                    rcol = rpool.tile([128, 1], F32, name="rcol", bufs=18)
                    nc.vector.reciprocal(out=rcol, in_=po[:, DH:DH + 1])
                    nc.vector.tensor_scalar_mul(
                        stages[sj][:, h, :], po[:, 0:DH], rcol
                    )

            def ctx_phase(j, q, xlns, us):
                """ctx accumulation (denominator row via the ones column) in
                two single-bank halves; transpose + normalization + stage."""
                h = 2 * j + q
                for n in range(2):
                    pc = ps_c.tile([DH + 1, 512], F32, name="pc")
                    for i in range(NT):
                        nc.tensor.matmul(
                            pc,
                            lhsT=xlns[q][:, i, :],
                            rhs=us[i][:, n * 512:(n + 1) * 512],
                            start=(i == 0),
                            stop=(i == NT - 1),
                        )
                    ctx_epilogue(h, n, pc)

            def flush(h0, h1):
                for sj in range(NT):
                    nc.sync.dma_start(
                        out=out_d[sj * 128:(sj + 1) * 128, h0 * 64:h1 * 64],
                        in_=stages[sj][:, h0:h1, :],
                    )

            state = prep(0)
            # PE warm-ups: fill idle cycles while inputs stream in so the
            # HAM clock gate opens before the first real matmuls.
            for _ in range(56):
                ptw = ps_t.tile([128, 128], BF16, name="pt")
                nc.tensor.transpose(ptw, identbf, identbf)
            pending = None  # (j, q, xlns, us) awaiting its ctx phase
            done_heads = 0
            for j in range(NP):
                zs, xlns = state
                for q in range(2):
                    last = (j == NP - 1 and q == 1)
                    if last:
                        # Final head: drain the pending ctx first (pool slot
                        # order), then interleave this head's ctx with its
                        # own scores so only the epilogue trails the last exp.
                        ctx_phase(*pending)
                        pending = None
                        flush(6, 10)
                        pcs = [ps_c.tile([DH + 1, 512], F32, name="pc"),
                               ps_s.tile([DH + 1, 512], F32, name="pm", bufs=1)]
                        scores_phase(j, q, zs, xlns, pcs)
                        for n in range(2):
                            ctx_epilogue(2 * j + q, n, pcs[n])
                        flush(10, 12)
                        continue
                    us = scores_phase(j, q, zs)
                    if pending is not None:
                        ctx_phase(*pending)
                        done_heads += 1
                        if done_heads == 6:
                            flush(0, 6)
                    pending = (j, q, xlns, us)
                    if q == 1:
                        state = prep(j + 1) if j + 1 < NP else None

    nc.compile()
    return nc


def kernel(x, attention_mask, W, b, _profile=None):
    global _CACHED_NC
    if _CACHED_NC is None:
        _CACHED_NC = build_nc()
    nc = _CACHED_NC

    x = np.asarray(x, dtype=np.float32)
    attention_mask = np.asarray(attention_mask, dtype=np.float32)
    W = np.asarray(W, dtype=np.float32)
    b = np.asarray(b, dtype=np.float32)

    import ml_dtypes

    wT = np.ascontiguousarray(W.T).astype(ml_dtypes.bfloat16)
    bias_cols = np.ascontiguousarray(b.reshape(NP, 128).T)

    in_maps = []
    for i in range(B):
        in_maps.append({
            "xT": np.ascontiguousarray(x[i].T).astype(ml_dtypes.bfloat16),
            "wT": wT,
            "bias_d": bias_cols,
            "mask_d": np.ascontiguousarray(
                attention_mask[i, 0, 0].reshape(NT, 128).T
            ),
        })

    kwargs = dict(_profile) if _profile else {}
    res = run_bass_kernel_spmd(nc, in_maps, core_ids=list(range(B)), **kwargs)
    out = np.stack([res.results[i]["out"] for i in range(B)], axis=0)
    if _profile:
        kernel.last_results = res
    return out


if __name__ == "__main__":
    rng = np.random.default_rng(0)
    x = rng.standard_normal((B, S, D), dtype=np.float32)
    m = np.zeros((B, 1, 1, S), dtype=np.float32)
    W = (rng.standard_normal((D, D), dtype=np.float32) / np.sqrt(D)).astype(np.float32)
    b = np.zeros((D,), dtype=np.float32)
    out = kernel(x, m, W, b)
    print("out", out.shape, out.dtype)


# revision 20
# speedup vs baseline: 1.0136x; 1.0136x over previous
"""BERT self-attention (B=8, S=1024, D=768, H=12) on 8 TRN2 NeuronCores.

Strategy
--------
Data-parallel over batch: core b handles batch element b (no collectives).

Per core, everything is computed in a "transposed" layout that keeps the
tensor engine's contraction dimension in the partition axis:

  1. mixedT[e, s] = sum_d W^T[d, e] * x^T[d, s] + bias[e] in fp32r matmuls
     (full fp32 inputs); the psum evacuation adds the per-partition bias
     and rounds to bf16 for the attention stage.
  2. Q=K=V => the score matrix is symmetric: scores[t, s] = scores[s, t].
     The exp'd score tile in [t, s] layout therefore equals the transposed
     (unnormalized) probability matrix needed as the moving operand of the
     context matmul -- no probability transposes at all.
     scores tile = (MIX chunk)^T @ Z_h where Z_h zero-masks the other head
     of the pair (kills cross-head terms, keeps K=128 partition-aligned
     bf16 matmuls at 1 cycle/column).
     U = exp(0.125 * scores + mask[t]) fused into the ACT psum evacuation;
     the instruction's accum_out gives the row sums, which (scores being
     symmetric and the attention mask identically zero per the problem
     spec) equal the softmax denominators for the matching s-chunk.
  3. ctx'^T[dh, s] accumulates in fp32 psum over the eight t-chunks with
     stationary xl [128, 64], moving U.
  4. PE-transpose of ctx'^T 128-column chunks gives ctx[s, dh]; the
     denominator reciprocal (computed early, off the critical path) is
     applied as a per-partition scalar during the psum evacuation; DMA to
     the output in natural [s, d] layout.

The pair loop is software-pipelined: the projection/prep work of pair j+1
is emitted right after the attention of pair j so the tensor engine fills
the bubbles of the ACT-paced softmax pipeline.
"""

import numpy as np

import concourse.bacc as bacc
import concourse.tile as tile
from concourse import mybir
from concourse.bass_utils import run_bass_kernel_spmd
from concourse.masks import make_identity

B, S, D = 8, 1024, 768
H, DH = 12, 64
NP = 6            # e-tile pairs (2 heads each)
NT = 8            # t-chunks / s-chunks of 128
F32 = mybir.dt.float32
F32R = mybir.dt.float32r
BF16 = mybir.dt.bfloat16
EXP = mybir.ActivationFunctionType.Exp

_CACHED_NC = None


def build_nc():
    nc = bacc.Bacc("TRN2", target_bir_lowering=False)

    xT = nc.dram_tensor("xT", [D, S], BF16, kind="ExternalInput")
    wT = nc.dram_tensor("wT", [D, D], BF16, kind="ExternalInput")
    bias_d = nc.dram_tensor("bias_d", [128, NP], F32, kind="ExternalInput")
    mask_d = nc.dram_tensor("mask_d", [128, NT], F32, kind="ExternalInput")
    out_d = nc.dram_tensor("out", [S, D], F32, kind="ExternalOutput")

    with tile.TileContext(nc) as tc:
        with (
            tc.tile_pool(name="consts", bufs=1) as consts,
            tc.tile_pool(name="big", bufs=1) as big,
            tc.tile_pool(name="upool", bufs=20) as upool,
            tc.tile_pool(name="ctpool", bufs=3) as ctpool,
            tc.tile_pool(name="rpool", bufs=18) as rpool,
            tc.tile_pool(name="ps_s", bufs=2, space="PSUM") as ps_s,
            tc.tile_pool(name="ps_c", bufs=1, space="PSUM") as ps_c,
            tc.tile_pool(name="ps_t", bufs=2, space="PSUM") as ps_t,
        ):
            ident32 = consts.tile([128, 128], F32)
            make_identity(nc, ident32)
            identbf = consts.tile([128, 128], BF16)
            make_identity(nc, identbf)
            wts = big.tile([128, NP, D], BF16)
            xts = big.tile([128, NP, S], BF16)
            for k in range(NP):
                eng = nc.sync if k % 2 == 0 else nc.scalar
                eng.dma_start(out=xts[:, k, :],
                              in_=xT[k * 128:(k + 1) * 128, :])
                nc.gpsimd.dma_start(out=wts[:, k, :],
                                    in_=wT[k * 128:(k + 1) * 128, :])
            bias_t = consts.tile([128, NP], F32)
            nc.gpsimd.dma_start(out=bias_t, in_=bias_d[:, :])
            mask_t = consts.tile([128, NT], F32)
            nc.gpsimd.dma_start(out=mask_t, in_=mask_d[:, :])

            # Preload the ACT exp table while the inputs stream in.
            warm = consts.tile([128, 16], F32)
            nc.scalar.activation(out=warm, in_=ident32[:, 0:16],
                                 func=EXP, scale=0.125)

            mixbf = big.tile([128, NP, S], BF16)
            stages = [big.tile([128, H, DH], F32, name=f"stage{sj}")
                      for sj in range(NT)]

            # Persistent ping-pong Z tiles; zero halves are set once.
            zt = [[big.tile([128, S], BF16, name=f"z{q}{p}") for p in range(2)]
                  for q in range(2)]
            xlt = [[big.tile([128, NT, DH + 1], BF16, name=f"xl{q}{p}")
                    for p in range(2)] for q in range(2)]
            for q in range(2):
                olo = (1 - q) * 64
                for p in range(2):
                    nc.vector.memset(zt[q][p][olo:olo + 64, :], 0.0)
                    nc.vector.memset(xlt[q][p], 1.0)

            def prep(j):
                """Projection + Z/xl staging for head pair j."""
                pp = j % 2
                if j == 0:
                    # Pair 0 is on the critical path: run both halves
                    # concurrently (second half borrows the idle ctx slot)
                    # so the projection tracks the input DMA arrivals.
                    pms = [ps_s.tile([128, 512], F32, name="pm", bufs=1),
                           ps_s.tile([128, 512], F32, name="psc")]
                    for k in range(NP):
                        for n in range(2):
                            nc.tensor.matmul(
                                pms[n],
                                lhsT=wts[:, k, j * 128:(j + 1) * 128],
                                rhs=xts[:, k, n * 512:(n + 1) * 512],
                                start=(k == 0),
                                stop=(k == NP - 1),
                            )
                    for n in range(2):
                        nc.vector.tensor_scalar_add(
                            mixbf[:, j, n * 512:(n + 1) * 512], pms[n],
                            bias_t[:, j:j + 1]
                        )
                else:
                    for n in range(2):
                        pm = ps_s.tile([128, 512], F32, name="pm", bufs=1)
                        for k in range(NP):
                            nc.tensor.matmul(
                                pm,
                                lhsT=wts[:, k, j * 128:(j + 1) * 128],
                                rhs=xts[:, k, n * 512:(n + 1) * 512],
                                start=(k == 0),
                                stop=(k == NP - 1),
                            )
                        nc.vector.tensor_scalar_add(
                            mixbf[:, j, n * 512:(n + 1) * 512], pm,
                            bias_t[:, j:j + 1]
                        )
                zs = []
                for q in range(2):
                    z = zt[q][pp]
                    lo = q * 64
                    for n in range(2):
                        nc.vector.tensor_copy(
                            out=z[lo:lo + 64, n * 512:(n + 1) * 512],
                            in_=mixbf[lo:lo + 64, j, n * 512:(n + 1) * 512],
                        )
                    zs.append(z)
                xlns = [xlt[0][pp], xlt[1][pp]]
                for i in range(NT):
                    pt = ps_t.tile([128, 128], BF16, name="pt")
                    nc.tensor.transpose(
                        pt, mixbf[:, j, i * 128:(i + 1) * 128], identbf
                    )
                    for q in range(2):
                        nc.vector.tensor_copy(
                            out=xlns[q][:, i, 0:DH], in_=pt[:, q * 64:q * 64 + 64]
                        )
                return zs, xlns

            def scores_phase(j, q, zs, xlns=None, pcs=None):
                """Scores + exp for head (j, q); returns the U tiles. When
                pcs is given (final head), the ctx matmuls are interleaved
                so only the epilogue remains after the last exp."""
                us = []
                for i in range(NT):
                    psc = ps_s.tile([128, S], F32, name="psc")
                    for n in range(2):
                        nc.tensor.matmul(
                            psc[:, n * 512:(n + 1) * 512],
                            lhsT=zs[q][:, i * 128:(i + 1) * 128],
                            rhs=mixbf[:, j, n * 512:(n + 1) * 512],
                            start=True,
                            stop=True,
                        )
                    u = upool.tile([128, S], BF16, name="u")
                    nc.scalar.activation(
                        out=u, in_=psc, func=EXP,
                        bias=mask_t[:, i:i + 1], scale=0.125,
                    )
                    us.append(u)
                    if pcs is not None:
                        for n in range(2):
                            nc.tensor.matmul(
                                pcs[n],
                                lhsT=xlns[q][:, i, :],
                                rhs=u[:, n * 512:(n + 1) * 512],
                                start=(i == 0),
                                stop=(i == NT - 1),
                            )
                return us

            def ctx_epilogue(h, n, pc):
                """Evacuate one ctx half: transpose + normalization + stage."""
                ct = ctpool.tile([DH + 1, 512], BF16, name="ct")
                nc.vector.tensor_copy(out=ct, in_=pc)
                for sjh in range(NT // 2):
                    sj = n * 4 + sjh
                    po = ps_t.tile([128, DH + 1], BF16, name="pt")
                    nc.tensor.transpose(
                        po,
                        ct[:, sjh * 128:(sjh + 1) * 128],
                        identbf[0:DH + 1, 0:DH + 1],
                    )
                    rcol = rpool.tile([128, 1], F32, name="rcol", bufs=18)
                    nc.vector.reciprocal(out=rcol, in_=po[:, DH:DH + 1])
                    nc.vector.tensor_scalar_mul(
                        stages[sj][:, h, :], po[:, 0:DH], rcol
                    )

            def ctx_phase(j, q, xlns, us):
                """ctx accumulation (denominator row via the ones column) in
                two single-bank halves; transpose + normalization + stage."""
                h = 2 * j + q
                for n in range(2):
                    pc = ps_c.tile([DH + 1, 512], F32, name="pc")
                    for i in range(NT):
                        nc.tensor.matmul(
                            pc,
                            lhsT=xlns[q][:, i, :],
                            rhs=us[i][:, n * 512:(n + 1) * 512],
                            start=(i == 0),
                            stop=(i == NT - 1),
                        )
                    ctx_epilogue(h, n, pc)

            def flush(h0, h1):
                for sj in range(NT):
                    nc.sync.dma_start(
                        out=out_d[sj * 128:(sj + 1) * 128, h0 * 64:h1 * 64],
                        in_=stages[sj][:, h0:h1, :],
                    )

            state = prep(0)
            pending = None  # (j, q, xlns, us) awaiting its ctx phase
            done_heads = 0
            for j in range(NP):
                zs, xlns = state
                for q in range(2):
                    last = (j == NP - 1 and q == 1)
                    if last:
                        # Final head: drain the pending ctx first (pool slot
                        # order), then interleave this head's ctx with its
                        # own scores so only the epilogue trails the last exp.
                        ctx_phase(*pending)
                        pending = None
                        flush(6, 10)
                        pcs = [ps_c.tile([DH + 1, 512], F32, name="pc"),
                               ps_s.tile([DH + 1, 512], F32, name="pm", bufs=1)]
                        scores_phase(j, q, zs, xlns, pcs)
                        for n in range(2):
                            ctx_epilogue(2 * j + q, n, pcs[n])
                        flush(10, 12)
                        continue
                    us = scores_phase(j, q, zs)
                    if pending is not None:
                        ctx_phase(*pending)
                        done_heads += 1
                        if done_heads == 6:
                            flush(0, 6)
                    pending = (j, q, xlns, us)
                    if q == 1:
                        state = prep(j + 1) if j + 1 < NP else None

    nc.compile()
    return nc


def kernel(x, attention_mask, W, b, _profile=None):
    global _CACHED_NC
    if _CACHED_NC is None:
        _CACHED_NC = build_nc()
    nc = _CACHED_NC

    x = np.asarray(x, dtype=np.float32)
    attention_mask = np.asarray(attention_mask, dtype=np.float32)
    W = np.asarray(W, dtype=np.float32)
    b = np.asarray(b, dtype=np.float32)

    import ml_dtypes

    wT = np.ascontiguousarray(W.T).astype(ml_dtypes.bfloat16)
    bias_cols = np.ascontiguousarray(b.reshape(NP, 128).T)

    in_maps = []
    for i in range(B):
        in_maps.append({
            "xT": np.ascontiguousarray(x[i].T).astype(ml_dtypes.bfloat16),
            "wT": wT,
            "bias_d": bias_cols,
            "mask_d": np.ascontiguousarray(
                attention_mask[i, 0, 0].reshape(NT, 128).T
            ),
        })

    kwargs = dict(_profile) if _profile else {}
    res = run_bass_kernel_spmd(nc, in_maps, core_ids=list(range(B)), **kwargs)
    out = np.stack([res.results[i]["out"] for i in range(B)], axis=0)
    if _profile:
        kernel.last_results = res
    return out


if __name__ == "__main__":
    rng = np.random.default_rng(0)
    x = rng.standard_normal((B, S, D), dtype=np.float32)
    m = np.zeros((B, 1, 1, S), dtype=np.float32)
    W = (rng.standard_normal((D, D), dtype=np.float32) / np.sqrt(D)).astype(np.float32)
    b = np.zeros((D,), dtype=np.float32)
    out = kernel(x, m, W, b)
    print("out", out.shape, out.dtype)
